# revision 1
# baseline (speedup 1.0000x reference)
"""Localized embedding layer (separable 5x5 Gaussian stencil) on 8 trn2 cores.

Math: out[i,j,:] = sum_{|di|<=2,|dj|<=2} w(di)w(dj) H[i+di,j+dj,:] / den(i,j)
with w(d) = exp(-c*d^2), c = TILE^2/(2 sigma^2), den(i,j) = r(i)*r(j) rank-1.

Per core (32 output grid rows + 2-row halo each side, zero padded):
  - i-conv (across grid rows)  -> DVE/GPSIMD: 4 fused ops per [128,2*512] row
  - j-conv (across partitions) -> TensorE: one 128x128 banded diag-block
    matmul per output half (1/(r(j)*W_full) folded in); the 4 output columns
    (j=126..129) whose stencil crosses the half boundary are recomputed by a
    batched fix pass: boundary slices of all 32 rows gathered into 2 tiles,
    2 block-diagonal matmuls, written out separately.
  - ScalarE: PSUM->SBUF copy with per-row scale W_full/r(i) (=1 in interior)
  - DMA out (main tiles skip the 4 fixed columns)
"""

import sys
import numpy as np

if "/opt/trn_rl_repo" not in sys.path:
    sys.path.insert(0, "/opt/trn_rl_repo")

G = 256          # grid side
D = 512          # feature dim
P = 2            # grid_step halo
NC = 8           # cores
RPC = G // NC    # rows per core = 32
TILE = 448.0
SIGMA = 200.0

_cache = {}


def _weights():
    c = TILE * TILE / (2.0 * SIGMA * SIGMA)
    return np.exp(-c * np.arange(-P, P + 1) ** 2)   # [w2,w1,1,w1,w2] f64


def _r_vec():
    """r(i) = sum of valid 1D taps at row i (same for columns)."""
    w = _weights()
    r = np.zeros(G)
    for d in range(-P, P + 1):
        lo, hi = max(0, -d), min(G, G - d)
        r[lo:hi] += w[d + P]
    return r


def _host_consts():
    w = _weights()
    r = _r_vec()
    w_full = w.sum()
    # Banded matrix Bp[jout, jin] = w(jout-jin) / (r(jout) * w_full)
    Bp = np.zeros((G, G))
    for d in range(-P, P + 1):
        for jout in range(G):
            jin = jout + d
            if 0 <= jin < G:
                Bp[jout, jin] = w[d + P] / (r[jout] * w_full)
    # main-pass lhsT: diagonal blocks only, layout [k, hm, m]
    wmat = np.zeros((128, 2, 128), dtype=np.float32)
    for hm in range(2):
        blk = Bp[128 * hm:128 * hm + 128, 128 * hm:128 * hm + 128]
        wmat[:, hm, :] = blk.T.astype(np.float32)
    # fix-pass i-conv lhsT [36, 32]: Tstrip[i] = sum_k w[k] * XS[i+k]
    wstrip = np.zeros((RPC + 2 * P, RPC), dtype=np.float32)
    for i in range(RPC):
        for k in range(5):
            wstrip[i + k, i] = w[k]
    # per-core scales
    scales, sfixes = [], []
    for c in range(NC):
        s = (w_full / r[RPC * c: RPC * (c + 1)]).astype(np.float32)
        scales.append(np.broadcast_to(s[None, :], (128, RPC)).copy())
        # strip scale: 1 / (r_i * w_full), per output row (partition)
        sf = np.zeros((128, 1), dtype=np.float32)
        sf[:RPC, 0] = (1.0 / (r[RPC * c: RPC * (c + 1)] * w_full)).astype(np.float32)
        sfixes.append(sf)
    return wmat, wstrip, scales, sfixes


def _build_nc(repeats=1):
    import concourse.bass as bass
    import concourse.mybir as mybir
    import concourse.tile as tile
    from concourse import bacc

    f32 = mybir.dt.float32
    add = mybir.AluOpType.add
    mult = mybir.AluOpType.mult

    w = _weights()
    w1, w2 = float(w[1]), float(w[0])
    NR = RPC + 2 * P

    nc = bacc.Bacc(None, target_bir_lowering=False, debug=False)
    x_dram = nc.declare_dram_parameter("x", [NR, G, D], f32, isOutput=False)
    wm_dram = nc.declare_dram_parameter("wmat", [128, 2, 128], f32, isOutput=False)
    wf_dram = nc.declare_dram_parameter("wstrip", [NR, RPC], f32, isOutput=False)
    sc_dram = nc.declare_dram_parameter("scale", [128, RPC], f32, isOutput=False)
    sf_dram = nc.declare_dram_parameter("sfix", [128, 1], f32, isOutput=False)
    y_dram = nc.declare_dram_parameter("y", [RPC, G, D], f32, isOutput=True)

    # rows whose t1 add goes to gpsimd (load balance: POOL ~38 of 64 adds)
    T1_POOL = {i for i in range(RPC) if i % 5 == 0}

    with tile.TileContext(nc) as tc:
        with (
            tc.tile_pool(name="const", bufs=1) as cpool,
            tc.tile_pool(name="x", bufs=8) as xpool,
            tc.tile_pool(name="tmp", bufs=3) as tpool,
            tc.tile_pool(name="tacc", bufs=4) as tapool,
            tc.tile_pool(name="out", bufs=6) as opool,
            tc.tile_pool(name="fix", bufs=1) as fpool,
            tc.tile_pool(name="psum", bufs=4, space="PSUM") as ppool,
            tc.tile_pool(name="psfix", bufs=2, space="PSUM") as pfpool,
        ):
            wt = cpool.tile([128, 2, 128], f32)
            nc.sync.dma_start(wt[:], wm_dram[:])
            wft = cpool.tile([NR, RPC], f32)
            nc.sync.dma_start(wft[:], wf_dram[:])
            st = cpool.tile([128, RPC], f32)
            nc.sync.dma_start(st[:], sc_dram[:])
            sft = cpool.tile([128, 1], f32)
            nc.sync.dma_start(sft[:], sf_dram[:])

            xt = {}

            def load_row(r):
                t = xpool.tile([128, 2, D], f32, tag="xrow")
                nc.sync.dma_start(
                    t[:], x_dram[r % NR].rearrange("(h p) d -> p h d", p=128)
                )
                xt[r] = t

            for r in range(5):
                load_row(r)

            for rep in range(repeats):
                for i in range(RPC):
                    it = rep * RPC + i
                    if it > 0:
                        load_row(it + 4)
                    a0, a1, a2, a3, a4 = (xt[it + k][:, :, :] for k in range(5))
                    t1 = tpool.tile([128, 2, D], f32, tag="t1")
                    eng1 = nc.gpsimd if i in T1_POOL else nc.vector
                    eng1.tensor_tensor(t1[:], a1, a3, add)
                    t2 = tpool.tile([128, 2, D], f32, tag="t2")
                    nc.gpsimd.tensor_tensor(t2[:], a0, a4, add)
                    t3 = tpool.tile([128, 2, D], f32, tag="t3")
                    nc.vector.scalar_tensor_tensor(t3[:], t2[:], w2 / w1, t1[:], mult, add)
                    tt = tapool.tile([128, 2, D], f32, tag="tacc")
                    nc.vector.scalar_tensor_tensor(tt[:], t3[:], w1, a2, mult, add)
                    for hm in range(2):
                        ps = ppool.tile([128, D], f32, tag="ps")
                        nc.tensor.matmul(
                            ps[:], wt[:, hm, :], tt[:, hm, :], start=True, stop=True
                        )
                        ob = opool.tile([128, D], f32, tag="ob")
                        nc.scalar.mul(ob[:], ps[:], st[:, i:i + 1])
                        if hm == 0:
                            nc.sync.dma_start(y_dram[i, 0:126, :], ob[0:126, :])
                        else:
                            nc.sync.dma_start(y_dram[i, 130:256, :], ob[2:128, :])
                # strip fix pass: recompute jout 126..129 for all 32 rows.
                # XS[r, jj, d] = x[r, 124+jj, d]  (jin strip), partition = row
                xs = fpool.tile([NR, 8, D], f32, tag="xs")
                nc.sync.dma_start(xs[:], x_dram[:, 124:132, :])
                # i-conv on PE: TS[i, jj, d] = sum_k w[k] XS[i+k, jj, d]
                ts = fpool.tile([RPC, 8, D], f32, tag="ts")
                for nchunk in range(8):
                    psf = pfpool.tile([RPC, D], f32, tag="psf")
                    nc.tensor.matmul(
                        psf[:], wft[:], xs[:, nchunk, :], start=True, stop=True
                    )
                    nc.scalar.copy(ts[:, nchunk, :], psf[:])
                # j-conv on free-dim shifts of TS (jout 126..129 <- jj slices)
                f1 = fpool.tile([RPC, 4, D], f32, tag="f1")
                nc.gpsimd.tensor_tensor(f1[:], ts[:, 1:5, :], ts[:, 3:7, :], add)
                f2 = fpool.tile([RPC, 4, D], f32, tag="f2")
                nc.gpsimd.tensor_tensor(f2[:], ts[:, 0:4, :], ts[:, 4:8, :], add)
                f3 = fpool.tile([RPC, 4, D], f32, tag="f3")
                nc.vector.scalar_tensor_tensor(f3[:], f2[:], w2 / w1, f1[:], mult, add)
                f4 = fpool.tile([RPC, 4, D], f32, tag="f4")
                nc.vector.scalar_tensor_tensor(f4[:], f3[:], w1, ts[:, 2:6, :], mult, add)
                fs = fpool.tile([RPC, 4, D], f32, tag="fs")
                nc.scalar.mul(fs[:], f4[:], sft[0:RPC, 0:1])
                nc.sync.dma_start(y_dram[:, 126:130, :], fs[:])
    nc.finalize()
    return nc


def _get_program():
    if "nc" not in _cache:
        _cache["nc"] = _build_nc()
        _cache["consts"] = _host_consts()
    return _cache["nc"], _cache["consts"]


def kernel(H, xy=None):
    from concourse.bass_utils import run_bass_kernel_spmd

    nc, (wmat, wstrip, scales, sfixes) = _get_program()
    H3 = np.ascontiguousarray(H.reshape(G, G, D).astype(np.float32))
    Hp = np.zeros((G + 2 * P, G, D), dtype=np.float32)
    Hp[P:P + G] = H3
    in_maps = []
    for c in range(NC):
        shard = np.ascontiguousarray(Hp[RPC * c: RPC * c + RPC + 2 * P])
        in_maps.append(
            {"x": shard, "wmat": wmat, "wstrip": wstrip,
             "scale": scales[c], "sfix": sfixes[c]}
        )
    res = run_bass_kernel_spmd(nc, in_maps, list(range(NC))).results
    out = np.concatenate([res[c]["y"].reshape(RPC * G, D) for c in range(NC)], axis=0)
    return out



# revision 3
# speedup vs baseline: 2.8780x; 2.8780x over previous
"""Localized embedding layer (Gaussian stencil) on 8 trn2 cores — bf16 pipeline.

Math: out[i,j,:] = sum_{|di|,|dj|<=2} w(di)w(dj) H[i+di,j+dj,:] / (r(i)*r(j))
with w(d) = exp(-c*d^2), c = TILE^2/(2 sigma^2). w(2) ~ 4.4e-5 is far below
the error budget, so the numerator collapses to a 3x3 stencil (the r(i)r(j)
normalizer keeps the exact 5-tap sums of the reference).

Sharding: 32 grid rows per core + 1-row halo each side (zero padded at the
global edges). All device I/O is bf16 (~0.2% rms quantization, budget 2e-2),
which halves HBM traffic — this problem is memory-bound.

Layout: one grid row = one SBUF tile [128 partitions, 2 cells x 512 feat];
partition p holds cells 2p, 2p+1, so DRAM lines are 2KB+ contiguous (host
pre-transposes shards to partition-major [128, rows, 1024]).

Per output row:
  - i-conv on DVE (bf16 2x mode): t1 = x[i-1]+x[i+1]; tt = w1*t1 + x[i]
  - j-conv on PE: 4 bf16 matmuls [128x128]x[128,512] into one 2-bank PSUM
    tile; the 4 matrices (diag / off-diag per cell parity) carry the exact
    1/(w_full*r_col(j)) column normalizer, so grid-edge columns need no fix
    pass at all.
  - ACT: PSUM->SBUF copy with per-row scale w_full/r_row(i) (=1 interior),
    converting f32 -> bf16.
DMA in 5 chunks (1-2MB loads) / 4 chunks (2MB stores) for near-peak HBM
efficiency, all loads issued up front.
"""

import sys
import numpy as np

if "/opt/trn_rl_repo" not in sys.path:
    sys.path.insert(0, "/opt/trn_rl_repo")

G = 256          # grid side
D = 512          # feature dim
NC = 8           # cores
RPC = G // NC    # output rows per core = 32
NR = RPC + 2     # input rows per core incl 1-row halo = 34
G2 = 2 * D       # free size of one grid row tile (2 cells x 512)
TILE = 448.0
SIGMA = 200.0
P5 = 2           # reference stencil half-width (for the normalizer r)

_cache = {}


def _weights5():
    c = TILE * TILE / (2.0 * SIGMA * SIGMA)
    return np.exp(-c * np.arange(-P5, P5 + 1) ** 2)   # [w2,w1,1,w1,w2] f64


def _r_vec():
    """r(i) = sum of valid 5-tap weights at row/col i (reference normalizer)."""
    w = _weights5()
    r = np.zeros(G)
    for d in range(-P5, P5 + 1):
        lo, hi = max(0, -d), min(G, G - d)
        r[lo:hi] += w[d + P5]
    return r


def _host_consts():
    import ml_dtypes

    w = _weights5()
    u1 = float(w[1])
    r = _r_vec()
    wf = float(w.sum())
    # Column-normalized j-conv matrices, wmat[q, k, p] = M_k[q, p] where
    # ps_c[p] = sum_q M[q,p] * tt[q].  s0/s1 fold 1/(w_full * r_col).
    wmat = np.zeros((128, 4, 128), dtype=np.float64)
    for p in range(128):
        s0 = 1.0 / (wf * r[2 * p])
        s1 = 1.0 / (wf * r[2 * p + 1])
        wmat[p, 0, p] = s0                      # D0: c0 center
        wmat[p, 1, p] = u1 * s0                 # B: c0 <- c1 (cell 2p+1)
        if p > 0:
            wmat[p - 1, 1, p] = u1 * s0         # B: c0 <- c1 (cell 2p-1)
        wmat[p, 2, p] = u1 * s1                 # C: c1 <- c0 (cell 2p)
        if p < 127:
            wmat[p + 1, 2, p] = u1 * s1         # C: c1 <- c0 (cell 2p+2)
        wmat[p, 3, p] = s1                      # D1: c1 center
    wmat = wmat.astype(ml_dtypes.bfloat16)
    # Per-core per-row scale w_full / r_row (=1 away from the global edges).
    srows = []
    for c in range(NC):
        s = (wf / r[RPC * c: RPC * (c + 1)]).astype(np.float32)
        srows.append(np.broadcast_to(s[None, :], (128, RPC)).copy())
    return u1, wmat, srows


# input chunks (start_row, n_rows) in the 34-row shard; output chunk size
IN_CHUNKS = [(0, 6), (6, 4), (10, 8), (18, 8), (26, 8)]
OUT_CHUNK = 8


def _build_nc(u1):
    import concourse.bass as bass  # noqa: F401
    import concourse.mybir as mybir
    import concourse.tile as tile
    from concourse import bacc

    f32 = mybir.dt.float32
    bf16 = mybir.dt.bfloat16
    add = mybir.AluOpType.add
    mult = mybir.AluOpType.mult

    nc = bacc.Bacc(None, target_bir_lowering=False, debug=False)
    x_dram = nc.declare_dram_parameter("x", [128, NR, G2], bf16, isOutput=False)
    wm_dram = nc.declare_dram_parameter("wmat", [128, 4, 128], bf16, isOutput=False)
    sr_dram = nc.declare_dram_parameter("srow", [128, RPC], f32, isOutput=False)
    y_dram = nc.declare_dram_parameter("y", [128, RPC, G2], bf16, isOutput=True)

    with tile.TileContext(nc) as tc:
        with (
            tc.tile_pool(name="const", bufs=1) as cpool,
            tc.tile_pool(name="x", bufs=len(IN_CHUNKS)) as xpool,
            tc.tile_pool(name="t1p", bufs=4) as tpool,
            tc.tile_pool(name="ttp", bufs=4) as ttpool,
            tc.tile_pool(name="out", bufs=2) as opool,
            tc.tile_pool(name="psum", bufs=4, space="PSUM") as ppool,
        ):
            wt = cpool.tile([128, 4, 128], bf16)
            nc.sync.dma_start(wt[:], wm_dram[:])
            srt = cpool.tile([128, RPC], f32)
            nc.sync.dma_start(srt[:], sr_dram[:])

            rowtile = {}
            for s, ln in IN_CHUNKS:
                xc = xpool.tile([128, 8, G2], bf16, tag="xc", name="xc")
                nc.sync.dma_start(xc[:, 0:ln, :], x_dram[:, s:s + ln, :])
                for k in range(ln):
                    rowtile[s + k] = (xc, k)

            for ck in range(RPC // OUT_CHUNK):
                ob = opool.tile([128, OUT_CHUNK, G2], bf16, tag="ob", name="ob")
                for m in range(OUT_CHUNK):
                    i = ck * OUT_CHUNK + m
                    ta, ia = rowtile[i]
                    tb, ib = rowtile[i + 1]
                    tcn, ic = rowtile[i + 2]
                    t1 = tpool.tile([128, G2], bf16, tag="t1", name="t1")
                    nc.vector.tensor_tensor(t1[:], ta[:, ia, :], tcn[:, ic, :], add)
                    ttv = ttpool.tile([128, G2], bf16, tag="tt", name="ttv")
                    nc.vector.scalar_tensor_tensor(
                        ttv[:], t1[:], u1, tb[:, ib, :], mult, add
                    )
                    ps = ppool.tile([128, G2], f32, tag="ps", name="ps")
                    nc.tensor.matmul(
                        ps[:, 0:D], wt[:, 0, :], ttv[:, 0:D], start=True, stop=False
                    )
                    nc.tensor.matmul(
                        ps[:, 0:D], wt[:, 1, :], ttv[:, D:G2], start=False, stop=True
                    )
                    nc.tensor.matmul(
                        ps[:, D:G2], wt[:, 2, :], ttv[:, 0:D], start=True, stop=False
                    )
                    nc.tensor.matmul(
                        ps[:, D:G2], wt[:, 3, :], ttv[:, D:G2], start=False, stop=True
                    )
                    nc.scalar.mul(ob[:, m, :], ps[:], srt[:, i:i + 1])
                nc.sync.dma_start(
                    y_dram[:, ck * OUT_CHUNK:(ck + 1) * OUT_CHUNK, :], ob[:]
                )
    nc.finalize()
    return nc


def _get_program():
    if "nc" not in _cache:
        consts = _host_consts()
        _cache["consts"] = consts
        _cache["nc"] = _build_nc(consts[0])
    return _cache["nc"], _cache["consts"]


def _in_maps(H):
    import ml_dtypes

    _, (u1, wmat, srows) = _get_program()
    H3 = np.asarray(H, dtype=np.float32).reshape(G, G, D)
    Hp = np.zeros((G + 2, G, D), dtype=np.float32)
    Hp[1:G + 1] = H3
    in_maps = []
    for c in range(NC):
        shard = Hp[RPC * c: RPC * c + NR]                     # [34, 256, 512]
        xp = np.ascontiguousarray(
            shard.reshape(NR, 128, 2, D).transpose(1, 0, 2, 3)
        ).reshape(128, NR, G2).astype(ml_dtypes.bfloat16)
        in_maps.append({"x": xp, "wmat": wmat, "srow": srows[c]})
    return in_maps


def _gather(results):
    outs = []
    for c in range(NC):
        y = np.asarray(results[c]["y"]).reshape(128, RPC, 2, D)
        y = y.transpose(1, 0, 2, 3).reshape(RPC * G, D).astype(np.float32)
        outs.append(y)
    return np.concatenate(outs, axis=0)


def kernel(H, xy=None):
    from concourse.bass_utils import run_bass_kernel_spmd

    nc, _ = _get_program()
    res = run_bass_kernel_spmd(nc, _in_maps(H), list(range(NC))).results
    return _gather(res)


# revision 4
# speedup vs baseline: 2.9560x; 1.0271x over previous
"""Localized embedding layer (Gaussian stencil) on 8 trn2 cores — bf16 pipeline.

Math: out[i,j,:] = sum_{|di|,|dj|<=2} w(di)w(dj) H[i+di,j+dj,:] / (r(i)*r(j))
with w(d) = exp(-c*d^2), c = TILE^2/(2 sigma^2). w(2) ~ 4.4e-5 is far below
the error budget, so the numerator collapses to a 3x3 stencil (the r(i)r(j)
normalizer keeps the exact 5-tap sums of the reference).

Sharding: 32 grid rows per core + 1-row halo each side (zero padded at the
global edges). All device I/O is bf16 (~0.2% rms quantization, budget 2e-2),
which halves HBM traffic — this problem is memory-bound.

Layout: one grid row = one SBUF tile [128 partitions, 2 cells x 512 feat];
partition p holds cells 2p, 2p+1, so DRAM lines are 2KB+ contiguous (host
pre-transposes shards to partition-major [128, rows, 1024]).

Per output row:
  - i-conv on DVE (bf16 2x mode): t1 = x[i-1]+x[i+1]; tt = w1*t1 + x[i]
  - j-conv on PE: 4 bf16 matmuls [128x128]x[128,512] into one 2-bank PSUM
    tile; the 4 matrices (diag / off-diag per cell parity) carry the exact
    1/(w_full*r_col(j)) column normalizer, so grid-edge columns need no fix
    pass at all.
  - ACT: PSUM->SBUF copy with per-row scale w_full/r_row(i) (=1 interior),
    converting f32 -> bf16.
DMA in 5 chunks (1-2MB loads) / 4 chunks (2MB stores) for near-peak HBM
efficiency, all loads issued up front.
"""

import sys
import numpy as np

if "/opt/trn_rl_repo" not in sys.path:
    sys.path.insert(0, "/opt/trn_rl_repo")

G = 256          # grid side
D = 512          # feature dim
NC = 8           # cores
RPC = G // NC    # output rows per core = 32
NR = RPC + 2     # input rows per core incl 1-row halo = 34
G2 = 2 * D       # free size of one grid row tile (2 cells x 512)
TILE = 448.0
SIGMA = 200.0
P5 = 2           # reference stencil half-width (for the normalizer r)

_cache = {}


def _weights5():
    c = TILE * TILE / (2.0 * SIGMA * SIGMA)
    return np.exp(-c * np.arange(-P5, P5 + 1) ** 2)   # [w2,w1,1,w1,w2] f64


def _r_vec():
    """r(i) = sum of valid 5-tap weights at row/col i (reference normalizer)."""
    w = _weights5()
    r = np.zeros(G)
    for d in range(-P5, P5 + 1):
        lo, hi = max(0, -d), min(G, G - d)
        r[lo:hi] += w[d + P5]
    return r


def _host_consts():
    import ml_dtypes

    w = _weights5()
    u1 = float(w[1])
    r = _r_vec()
    wf = float(w.sum())
    # Column-normalized j-conv matrices, wmat[q, k, p] = M_k[q, p] where
    # ps_c[p] = sum_q M[q,p] * tt[q].  s0/s1 fold 1/(w_full * r_col).
    wmat = np.zeros((128, 4, 128), dtype=np.float64)
    for p in range(128):
        s0 = 1.0 / (wf * r[2 * p])
        s1 = 1.0 / (wf * r[2 * p + 1])
        wmat[p, 0, p] = s0                      # D0: c0 center
        wmat[p, 1, p] = u1 * s0                 # B: c0 <- c1 (cell 2p+1)
        if p > 0:
            wmat[p - 1, 1, p] = u1 * s0         # B: c0 <- c1 (cell 2p-1)
        wmat[p, 2, p] = u1 * s1                 # C: c1 <- c0 (cell 2p)
        if p < 127:
            wmat[p + 1, 2, p] = u1 * s1         # C: c1 <- c0 (cell 2p+2)
        wmat[p, 3, p] = s1                      # D1: c1 center
    wmat = wmat.astype(ml_dtypes.bfloat16)
    # Per-core per-row scale w_full / r_row (=1 away from the global edges).
    srows = []
    for c in range(NC):
        s = (wf / r[RPC * c: RPC * (c + 1)]).astype(np.float32)
        srows.append(np.broadcast_to(s[None, :], (128, RPC)).copy())
    return u1, wmat, srows


# 4 input chunks of 10 rows, stride 8 (2-row overlap so every 4-row packed
# DVE op stays within one tile); outputs stored per 4-row quad (1MB DMAs).
NCH = 4
CHR = 10
CHS = 8


def _build_nc(u1):
    import concourse.bass as bass  # noqa: F401
    import concourse.mybir as mybir
    import concourse.tile as tile
    from concourse import bacc

    f32 = mybir.dt.float32
    bf16 = mybir.dt.bfloat16
    add = mybir.AluOpType.add

    nc = bacc.Bacc(None, target_bir_lowering=False, debug=False)
    x_dram = nc.declare_dram_parameter("x", [128, NR, G2], bf16, isOutput=False)
    wm_dram = nc.declare_dram_parameter("wmat", [128, 4, 128], bf16, isOutput=False)
    sr_dram = nc.declare_dram_parameter("srow", [128, RPC], f32, isOutput=False)
    y_dram = nc.declare_dram_parameter("y", [128, RPC, G2], bf16, isOutput=True)

    with tile.TileContext(nc) as tc:
        with (
            tc.tile_pool(name="const", bufs=1) as cpool,
            tc.tile_pool(name="x", bufs=NCH) as xpool,
            tc.tile_pool(name="z", bufs=2) as zpool,
            tc.tile_pool(name="qp", bufs=2) as qpool,
            tc.tile_pool(name="ttp", bufs=2) as ttpool,
            tc.tile_pool(name="out", bufs=3) as opool,
            tc.tile_pool(name="psum", bufs=4, space="PSUM") as ppool,
        ):
            wt = cpool.tile([128, 4, 128], bf16)
            nc.sync.dma_start(wt[:], wm_dram[:])
            srt = cpool.tile([128, RPC], f32)
            nc.sync.dma_start(srt[:], sr_dram[:])

            xcs = []
            for j in range(NCH):
                xc = xpool.tile([128, CHR, G2], bf16, tag="xc", name="xc")
                nc.sync.dma_start(xc[:], x_dram[:, CHS * j:CHS * j + CHR, :])
                xcs.append(xc)

            for j in range(NCH):
                xc = xcs[j]
                # z = u1 * x for the whole chunk (DVE 4x tensor_scalar)
                zc = zpool.tile([128, CHR, G2], bf16, tag="zc", name="zc")
                nc.vector.tensor_scalar_mul(zc[:], xc[:], u1)
                for q in (2 * j, 2 * j + 1):          # quad = output rows 4q..4q+3
                    b = 4 * q - CHS * j               # chunk-local row offset
                    qt = qpool.tile([128, 4, G2], bf16, tag="qt", name="qt")
                    nc.vector.tensor_tensor(
                        qt[:], zc[:, b:b + 4, :], zc[:, b + 2:b + 6, :], add
                    )
                    ttq = ttpool.tile([128, 4, G2], bf16, tag="tt", name="ttq")
                    nc.vector.tensor_tensor(
                        ttq[:], qt[:], xc[:, b + 1:b + 5, :], add
                    )
                    ob = opool.tile([128, 4, G2], bf16, tag="ob", name="ob")
                    for m in range(4):
                        i = 4 * q + m
                        ps = ppool.tile([128, G2], f32, tag="ps", name="ps")
                        nc.tensor.matmul(
                            ps[:, 0:D], wt[:, 0, :], ttq[:, m, 0:D],
                            start=True, stop=False,
                        )
                        nc.tensor.matmul(
                            ps[:, 0:D], wt[:, 1, :], ttq[:, m, D:G2],
                            start=False, stop=True,
                        )
                        nc.tensor.matmul(
                            ps[:, D:G2], wt[:, 2, :], ttq[:, m, 0:D],
                            start=True, stop=False,
                        )
                        nc.tensor.matmul(
                            ps[:, D:G2], wt[:, 3, :], ttq[:, m, D:G2],
                            start=False, stop=True,
                        )
                        nc.scalar.mul(ob[:, m, :], ps[:], srt[:, i:i + 1])
                    # store this quad on the scalar engine's DMA ring so
                    # stores don't queue behind the remaining loads
                    nc.scalar.dma_start(
                        y_dram[:, 4 * q:4 * q + 4, :], ob[:]
                    )
    nc.finalize()
    return nc


def _get_program():
    if "nc" not in _cache:
        consts = _host_consts()
        _cache["consts"] = consts
        _cache["nc"] = _build_nc(consts[0])
    return _cache["nc"], _cache["consts"]


def _in_maps(H):
    import ml_dtypes

    _, (u1, wmat, srows) = _get_program()
    H3 = np.asarray(H, dtype=np.float32).reshape(G, G, D)
    Hp = np.zeros((G + 2, G, D), dtype=np.float32)
    Hp[1:G + 1] = H3
    in_maps = []
    for c in range(NC):
        shard = Hp[RPC * c: RPC * c + NR]                     # [34, 256, 512]
        xp = np.ascontiguousarray(
            shard.reshape(NR, 128, 2, D).transpose(1, 0, 2, 3)
        ).reshape(128, NR, G2).astype(ml_dtypes.bfloat16)
        in_maps.append({"x": xp, "wmat": wmat, "srow": srows[c]})
    return in_maps


def _gather(results):
    outs = []
    for c in range(NC):
        y = np.asarray(results[c]["y"]).reshape(128, RPC, 2, D)
        y = y.transpose(1, 0, 2, 3).reshape(RPC * G, D).astype(np.float32)
        outs.append(y)
    return np.concatenate(outs, axis=0)


def kernel(H, xy=None):
    from concourse.bass_utils import run_bass_kernel_spmd

    nc, _ = _get_program()
    res = run_bass_kernel_spmd(nc, _in_maps(H), list(range(NC))).results
    return _gather(res)
